# revision 1
# baseline (speedup 1.0000x reference)
"""Trainium2 Bass kernel for nn_CentersDistance (retrieval_knn).

logits[k, n] = -||centers[k] - inputs[n]||^2
             = 2*(centers @ inputs.T)[k, n] - ||centers[k]||^2 - ||inputs[n]||^2

Strategy (8 NeuronCores, data-parallel over the N=8192 inputs):
  * host: transpose both operands so the contraction dim D lands on the SBUF
    partition axis ([D, K] and [D, N/8] layouts), fold the factor 2 into the
    inputs, and precompute the norm terms exactly in float64.
  * device (per core): a 1024x1024x1024 matmul in bf16 with fp32 PSUM
    accumulation (bf16 streams 1 row/cycle on the PE vs 4 for fp32; the
    measured end-to-end error is absmax/scale 3.3e-4, resid_var 5.4e-9,
    because the exact norm terms dominate the logits).  The epilogue runs on
    the DVE: one scalar_tensor_tensor op adds -||c||^2 (per-partition scalar)
    and -||x||^2 (broadcast row read from a host-precomputed [128, N/8]
    tile), output stored fp32.
  * raw Block/semaphore implementation (not Tile): Tile's ~50 semaphores are
    not the issue (the NRT pre/postamble resets a fixed 51 per engine), but
    Tile adds its own ~6 us drain + clear-semaphores + barrier tail, and its
    scheduler cannot express the exact warmup/pacing we want.
  * the PE is kept continuously busy from ~1 us into the kernel by N_WU
    throwaway matmuls on an (uninitialized) scratch tile so the HAM clock
    gate is fully open (2.4 GHz) when the first real matmul issues; the
    warmup count is sized to bridge until the first ct/xt tile pair lands.
  * loads stream on two HW-DGE queues (Sync: xt, Scalar: ct) with one
    semaphore per d-tile pair: completions of equal-size DMAs are usually in
    issue order, but HBM contention from the other 7 cores can invert them,
    and a single shared counter would then let the PE read a tile that is
    not fully written (observed as a sporadic inf in the output).
  * pass 1 (m-tiles 0-3) runs d outermost so matmuls pace with the streaming
    loads across 8 concurrent PSUM banks; pass 2 (m-tiles 4-7) runs d
    innermost so each output group retires early and its epilogue + store
    overlap the remaining matmuls.

Measured on 8 axon-tunneled trn2 cores: ~45 us NEFF exec (NTFF), of which
~27.6 us is the bf16 PE-stream floor (128 matmuls x 512 rows @ 2.4 GHz) and
~14 us is fixed NRT preamble/postamble (sync barriers, 51-semaphore reset
chains, dma_rearm).

A float32r variant (dt=mybir.dt.float32r, np_dt=np.float32) measures
~56 us / absmax 2.0e-5 — load-bound (8.5 MB vs 4.5 MB of input) but with
near-fp32 precision; kept as a fallback should tighter accuracy ever be
needed.  An fp8e4m3 DoubleRow variant measured ~36 us / absmax 5.2e-3 —
rejected for accuracy-risk reasons.
"""

import threading
from contextlib import ExitStack

import numpy as np
import ml_dtypes

import concourse.mybir as mybir
from concourse import bacc
from concourse.bass_utils import run_bass_kernel_spmd

N_CORES = 8
N, K, D = 8192, 1024, 1024
NSH = N // N_CORES  # per-core slab of inputs
P = 128             # SBUF partitions
NF = 512            # matmul moving free dim (one fp32 PSUM bank)

D_TILES = D // P    # 8 contraction tiles
M_TILES = K // P    # 8 center tiles
H_TILES = NSH // NF # 2 moving-dim tiles

G = M_TILES * H_TILES  # 16 output groups of [128, 512]
GP1 = 8                # groups 0-7 -> pass 1 (m-tiles 0-3), banks 0-7
N_WU = 10              # PE warm-up matmuls

_DT = mybir.dt.bfloat16
_NP_DT = ml_dtypes.bfloat16

_cache = threading.local()


def _g_mh(g):
    return g // H_TILES, g % H_TILES


def _build_nc(dt=_DT):
    nc = bacc.Bacc(
        "TRN2", target_bir_lowering=False, debug=False, num_devices=N_CORES
    )
    ct = nc.dram_tensor("ct", [D, K], dt, kind="ExternalInput").ap()
    xt = nc.dram_tensor("xt", [D, NSH], dt, kind="ExternalInput").ap()
    ncsq = nc.dram_tensor(
        "ncsq", [P, M_TILES], mybir.dt.float32, kind="ExternalInput"
    ).ap()
    nxsq = nc.dram_tensor(
        "nxsq", [P, NSH], mybir.dt.float32, kind="ExternalInput"
    ).ap()
    out = nc.dram_tensor("out", [K, NSH], mybir.dt.float32, kind="ExternalOutput").ap()

    ct_r = ct.rearrange("(t p) k -> t p k", p=P)
    xt_r = xt.rearrange("(t p) n -> t p n", p=P)
    out_r = out.rearrange("(m p) n -> m p n", p=P)

    HNF = NF // 2

    with (
        nc.sbuf_tensor("wu_sb", [P, NF], dt) as wu_sb,
        nc.sbuf_tensor("ncsq_sb", [P, M_TILES], mybir.dt.float32) as ncsq_sb,
        nc.sbuf_tensor("nxsq_sb", [P, NSH], mybir.dt.float32) as nxsq_sb,
        nc.sbuf_tensor("ot_sb", [P, G * NF], mybir.dt.float32) as ot_sb,
        ExitStack() as stack,
        nc.semaphore("const_sem") as const_sem,
        nc.semaphore("mm_sem") as mm_sem,
        nc.semaphore("dve_sem") as dve_sem,
        nc.semaphore("dma_out") as dma_out,
        nc.Block() as block,
    ):
        d_sems = [
            stack.enter_context(nc.semaphore(f"d_sem{i}")) for i in range(D_TILES)
        ]
        ct_sb = [
            stack.enter_context(nc.sbuf_tensor(f"ct_sb{d}", [P, K], dt))
            for d in range(D_TILES)
        ]
        xt_sb = [
            stack.enter_context(nc.sbuf_tensor(f"xt_sb{d}", [P, NSH], dt))
            for d in range(D_TILES)
        ]
        ps = [
            stack.enter_context(nc.psum_tensor(f"ps{b}", [P, NF], mybir.dt.float32))
            for b in range(8)
        ]

        @block.sync
        def _(sync):
            # xt on the Sync HW-DGE queue; ct goes out in parallel on the
            # Scalar engine's queue (block.scalar below) — two rings halve
            # the time to the first d-tile pair and keep the d-loop ahead
            # of the PE throughout
            for d in range(D_TILES):
                sync.dma_start(xt_sb[d][:], xt_r[d]).then_inc(d_sems[d], 16)
            # consts last: only the DVE epilogue (which runs late) needs them
            sync.dma_start(ncsq_sb[:], ncsq).then_inc(const_sem, 16)
            sync.dma_start(nxsq_sb[:], nxsq).then_inc(const_sem, 16)
            for g in range(G - 1):
                m, h = _g_mh(g)
                sync.wait_ge(dve_sem, g + 1)
                sync.dma_start(
                    out_r[m][:, h * NF : (h + 1) * NF],
                    ot_sb[:, g * NF : (g + 1) * NF],
                ).then_inc(dma_out, 16)
            # last group is split in half so its store starts while the DVE
            # is still draining the second half; the second half goes out on
            # the Scalar ring (see block.scalar) so the two final stores
            # complete in parallel — both are on the kernel's critical tail
            m, h = _g_mh(G - 1)
            sync.wait_ge(dve_sem, G)
            sync.dma_start(
                out_r[m][:, h * NF : h * NF + HNF],
                ot_sb[:, (G - 1) * NF : (G - 1) * NF + HNF],
            ).then_inc(dma_out, 16)
            sync.wait_ge(dma_out, (G + 1) * 16)

        @block.scalar
        def _(scalar):
            for d in range(D_TILES):
                scalar.dma_start(ct_sb[d][:], ct_r[d]).then_inc(d_sems[d], 16)
            m, h = _g_mh(G - 1)
            scalar.wait_ge(dve_sem, G + 1)
            scalar.dma_start(
                out_r[m][:, h * NF + HNF : (h + 1) * NF],
                ot_sb[:, (G - 1) * NF + HNF : G * NF],
            ).then_inc(dma_out, 16)

        @block.tensor
        def _(tensor):
            # warm-up: open the HAM clock gate while the loads stream.
            # wu_sb is deliberately uninitialized — the products are never
            # read, only the PE-busy time matters.  Bank 7 is rewritten with
            # start=True by group 7's first matmul ~8 matmuls later, long
            # after the last warmup has drained.
            for _ in range(N_WU):
                nc.tensor.matmul(
                    ps[GP1 - 1][:], wu_sb[:, 0:P], wu_sb[:], start=True, stop=True
                )
            # pass 1: groups 0-7 accumulate in banks 0-7, d outermost so
            # matmuls pace with the streaming loads
            for d in range(D_TILES):
                tensor.wait_ge(d_sems[d], 32)
                for g in range(GP1):
                    m, h = _g_mh(g)
                    mm = nc.tensor.matmul(
                        ps[g][:],
                        ct_sb[d][:, m * P : (m + 1) * P],
                        xt_sb[d][:, h * NF : (h + 1) * NF],
                        start=(d == 0),
                        stop=(d == D_TILES - 1),
                    )
                    if d == D_TILES - 1:
                        mm.then_inc(mm_sem, 1)
            # pass 2: groups 8-15 reuse banks 0-7 once the DVE epilogue has
            # drained the pass-1 group from that bank (P10: concurrent
            # PE-write + DVE-read of one PSUM bank is fatal, so this wait is
            # load-bearing, not just WAR ordering)
            for g in range(GP1, G):
                m, h = _g_mh(g)
                if g >= 8:
                    # bank g%8 was last drained by the DVE for group g-8
                    tensor.wait_ge(dve_sem, g - 8 + 1)
                for d in range(D_TILES):
                    mm = nc.tensor.matmul(
                        ps[g % 8][:],
                        ct_sb[d][:, m * P : (m + 1) * P],
                        xt_sb[d][:, h * NF : (h + 1) * NF],
                        start=(d == 0),
                        stop=(d == D_TILES - 1),
                    )
                mm.then_inc(mm_sem, 1)

        @block.vector
        def _(vector):
            vector.wait_ge(const_sem, 32)  # ncsq + nxsq present
            for g in range(G - 1):
                m, h = _g_mh(g)
                vector.wait_ge(mm_sem, g + 1)
                nc.vector.scalar_tensor_tensor(
                    ot_sb[:, g * NF : (g + 1) * NF],
                    ps[g % 8][:],
                    ncsq_sb[:, m : m + 1],
                    nxsq_sb[:, h * NF : (h + 1) * NF],
                    op0=mybir.AluOpType.add,
                    op1=mybir.AluOpType.add,
                ).then_inc(dve_sem, 1)
            m, h = _g_mh(G - 1)
            vector.wait_ge(mm_sem, G)
            for half in range(2):
                nc.vector.scalar_tensor_tensor(
                    ot_sb[
                        :,
                        (G - 1) * NF + half * HNF : (G - 1) * NF + (half + 1) * HNF,
                    ],
                    ps[(G - 1) % 8][:, half * HNF : (half + 1) * HNF],
                    ncsq_sb[:, m : m + 1],
                    nxsq_sb[:, h * NF + half * HNF : h * NF + (half + 1) * HNF],
                    op0=mybir.AluOpType.add,
                    op1=mybir.AluOpType.add,
                ).then_inc(dve_sem, 1)

    nc.compile()
    return nc


def _get_nc():
    if not hasattr(_cache, "nc"):
        _cache.nc = _build_nc()
    return _cache.nc


def kernel(inputs, centers, _trace=False, _np_dt=None):
    np_dt = _np_dt if _np_dt is not None else _NP_DT
    inputs = np.asarray(inputs, dtype=np.float32)
    centers = np.asarray(centers, dtype=np.float32)

    csq = np.sum(centers.astype(np.float64) ** 2, axis=1)
    xsq = np.sum(inputs.astype(np.float64) ** 2, axis=1)

    ct = np.ascontiguousarray(centers.T).astype(np_dt)
    xt2 = np.ascontiguousarray((2.0 * inputs).T.astype(np_dt))
    ncsq = np.ascontiguousarray((-csq).reshape(M_TILES, P).T.astype(np.float32))

    in_maps = []
    for i in range(N_CORES):
        sl = slice(i * NSH, (i + 1) * NSH)
        in_maps.append(
            {
                "ct": ct,
                "xt": np.ascontiguousarray(xt2[:, sl]),
                "ncsq": ncsq,
                "nxsq": np.ascontiguousarray(
                    np.broadcast_to(-xsq[sl], (P, NSH))
                ).astype(np.float32),
            }
        )

    nc = _get_nc()
    try:
        res = run_bass_kernel_spmd(
            nc, in_maps, core_ids=list(range(N_CORES)), trace=_trace
        )
    except ModuleNotFoundError:
        # NTFF trace glue is absent in some images; rerun without tracing
        res = run_bass_kernel_spmd(
            nc, in_maps, core_ids=list(range(N_CORES)), trace=False
        )
    if _trace:
        kernel.last_results = res
    return np.concatenate([r["out"] for r in res.results], axis=1)



# revision 8
# speedup vs baseline: 1.2386x; 1.2386x over previous
"""Trainium2 Bass kernel for nn_CentersDistance (retrieval_knn).

logits[k, n] = -||centers[k] - inputs[n]||^2
             = 2*(centers @ inputs.T)[k, n] - ||centers[k]||^2 - ||inputs[n]||^2

Strategy (8 NeuronCores, data-parallel over the N=8192 inputs):
  * host: transpose both operands so the contraction dim D lands on the SBUF
    partition axis, fold the factor 2 into the inputs, quantize both to
    fp8e4m3, and precompute the norm terms exactly in float64.
  * device (per core): a 1024x1024x1024 matmul in fp8 DoubleRow mode
    (2 contraction rows/cycle on the PE = 157 TF/s, 2x the bf16 rate).
    DoubleRow packs two contraction sub-rows per partition: operands are
    laid out [128, 2, free] per 256-deep d-super-tile (4 tiles cover
    D=1024), so the whole per-core product is 64 matmuls x 512 moving
    rows = 32768 PE cycles = 13.7 us of PE stream.
  * epilogue adds the exact norm terms (-csq per-partition scalar, -xsq
    broadcast row) with scalar_tensor_tensor, split across the DVE
    (even groups) and Pool/GpSimd (odd groups) engines so the ~0.7 us
    per-group PSUM-read cost never becomes the critical path; output is
    written fp16 (the norm terms dominate the logits, measured absmax
    error stays ~5e-3 of scale) and upconverted to fp32 on the host.
    fp16 stores also halve the output DMA traffic: all queues share the
    same 16 DMA engines (~368 GB/s per core total), so with fp8 loads
    (2 MB) + fp16 stores (2 MB) + norm tiles the total DMA time stays
    under the PE stream time.
  * raw Block/semaphore implementation (not Tile), same skeleton as the
    earlier bf16 version: PE warmup matmuls open the HAM clock gate
    while loads stream; pass 1 (m-tiles 0-3) runs d outermost to pace
    with the streaming loads across 8 PSUM banks; pass 2 (m-tiles 4-7)
    runs d innermost so groups retire early and their epilogue + store
    overlap the remaining matmuls.  Bank reuse in pass 2 waits on the
    corresponding epilogue (concurrent PE-write + DVE-read of a PSUM
    bank is fatal on P10).
  * loads stream on two HW-DGE queues (Sync: xt, Scalar: ct) with one
    semaphore per d-tile pair; the norm tiles ride the GpSimd queue.
    Stores go out one m-tile (two groups, [128, 1024] fp16 = 256 KB) at
    a time, even m on Sync, odd m on Scalar, final m-tile split so the
    two halves land on both queues.

Previous bf16 version measured 44.2 us NEFF exec; ~27.6 us of that was
the bf16 PE-stream floor, plus ~8.5 us fixed NRT pre/postamble
(51-semaphore reset chains per engine) that this version keeps paying.
"""

import threading
from contextlib import ExitStack

import numpy as np
import ml_dtypes

import concourse.mybir as mybir
from concourse import bacc
from concourse.bass_utils import run_bass_kernel_spmd

N_CORES = 8
N, K, D = 8192, 1024, 1024
NSH = N // N_CORES  # per-core slab of inputs
P = 128             # SBUF partitions
NF = 512            # matmul moving free dim (one fp32 PSUM bank)

DR = 2              # DoubleRow: contraction sub-rows per partition
DT_SUPER = P * DR   # 256 contraction rows per d-super-tile
D_TILES = D // DT_SUPER  # 4 contraction super-tiles
M_TILES = K // P    # 8 center tiles
H_TILES = NSH // NF # 2 moving-dim tiles

G = M_TILES * H_TILES  # 16 output groups of [128, 512]
GP1 = 8                # groups 0-7 -> pass 1 (m-tiles 0-3), banks 0-7
N_WU = 10              # PE warm-up matmuls

_DT = mybir.dt.float8e4
_NP_DT = ml_dtypes.float8_e4m3
_OUT_DT = mybir.dt.float16

_cache = threading.local()


def _g_mh(g):
    return g // H_TILES, g % H_TILES


def _build_nc():
    nc = bacc.Bacc(
        "TRN2", target_bir_lowering=False, debug=False, num_devices=N_CORES
    )
    # host pre-interleaved DoubleRow layouts: [t, p, (i, free)] with
    # logical contraction index d = t*256 + i*128 + p
    ct = nc.dram_tensor("ct", [D_TILES, P, DR * K], _DT, kind="ExternalInput").ap()
    xt = nc.dram_tensor("xt", [D_TILES, P, DR * NSH], _DT, kind="ExternalInput").ap()
    ncsq = nc.dram_tensor(
        "ncsq", [P, M_TILES], mybir.dt.float32, kind="ExternalInput"
    ).ap()
    nxsq = nc.dram_tensor(
        "nxsq", [P, NSH], mybir.dt.float32, kind="ExternalInput"
    ).ap()
    out = nc.dram_tensor("out", [K, NSH], _OUT_DT, kind="ExternalOutput").ap()

    out_r = out.rearrange("(m p) n -> m p n", p=P)

    with (
        nc.sbuf_tensor("wu_sb", [P, DR, NF], _DT) as wu_sb,
        nc.sbuf_tensor("ncsq_sb", [P, M_TILES], mybir.dt.float32) as ncsq_sb,
        nc.sbuf_tensor("nxsq_sb", [P, NSH], mybir.dt.float32) as nxsq_sb,
        nc.sbuf_tensor("ot_sb", [P, G * NF], _OUT_DT) as ot_sb,
        # double-buffered staging for the Act->GpSimd even-group pipeline
        nc.sbuf_tensor("tmp_sb", [P, 2 * NF], _OUT_DT) as tmp_sb,
        ExitStack() as stack,
        nc.semaphore("const_sem") as const_sem,
        nc.semaphore("mm_sem") as mm_sem,
        nc.semaphore("v_sem") as v_sem,   # odd-group epilogues (DVE, from PSUM)
        nc.semaphore("a_sem") as a_sem,   # even-group PSUM->SBUF copies (Act)
        nc.semaphore("g_sem") as g_sem,   # even-group epilogues (GpSimd, from SBUF)
        nc.semaphore("dma_out") as dma_out,
        nc.Block() as block,
    ):
        d_sems = [
            stack.enter_context(nc.semaphore(f"d_sem{i}")) for i in range(D_TILES)
        ]
        ct_sb = [
            stack.enter_context(nc.sbuf_tensor(f"ct_sb{d}", [P, DR, K], _DT))
            for d in range(D_TILES)
        ]
        xt_sb = [
            stack.enter_context(nc.sbuf_tensor(f"xt_sb{d}", [P, DR, NSH], _DT))
            for d in range(D_TILES)
        ]
        ps = [
            stack.enter_context(nc.psum_tensor(f"ps{b}", [P, NF], mybir.dt.float32))
            for b in range(8)
        ]

        def epi_wait(engine, g):
            """Wait until the epilogue for group g has drained its bank."""
            if g % 2 == 0:
                engine.wait_ge(a_sem, g // 2 + 1)
            else:
                engine.wait_ge(v_sem, g // 2 + 1)

        @block.sync
        def _(sync):
            for d in range(D_TILES):
                sync.dma_start(xt_sb[d][:], xt[d]).then_inc(d_sems[d], 16)
            # stores: one m-tile = groups 2m, 2m+1 = [128, 1024] fp16
            for m in (0, 2, 4, 6):
                sync.wait_ge(g_sem, m + 1)
                sync.wait_ge(v_sem, m + 1)
                sync.dma_start(
                    out_r[m], ot_sb[:, 2 * m * NF : (2 * m + 2) * NF]
                ).then_inc(dma_out, 16)
            # m=7 h=0 (group 14, even) as soon as its epilogue lands
            sync.wait_ge(g_sem, 8)
            sync.dma_start(
                out_r[7][:, 0:NF], ot_sb[:, 14 * NF : 15 * NF]
            ).then_inc(dma_out, 16)
            sync.wait_ge(dma_out, 9 * 16)

        @block.scalar
        def _(scalar):
            for d in range(D_TILES):
                scalar.dma_start(ct_sb[d][:], ct[d]).then_inc(d_sems[d], 16)
            # even-group epilogue stage 1: Act drains the PSUM bank into
            # fp16 staging (GpSimd cannot read PSUM on TRN2); the odd-m
            # stores are interleaved in expected-readiness order so they
            # are not program-order-blocked behind late copies
            def copy_j(j):
                g = 2 * j
                m, _ = _g_mh(g)
                scalar.wait_ge(mm_sem, g + 1)
                if j >= 2:
                    scalar.wait_ge(g_sem, j - 1)  # staging slot free
                # drain the bank adding the per-partition -csq on the way
                nc.scalar.activation(
                    tmp_sb[:, (j % 2) * NF : (j % 2 + 1) * NF],
                    ps[g % 8][:],
                    mybir.ActivationFunctionType.Identity,
                    bias=ncsq_sb[:, m : m + 1],
                    scale=1.0,
                ).then_inc(a_sem, 1)

            def store_m(m):
                scalar.wait_ge(g_sem, m + 1)
                scalar.wait_ge(v_sem, m + 1)
                scalar.dma_start(
                    out_r[m], ot_sb[:, 2 * m * NF : (2 * m + 2) * NF]
                ).then_inc(dma_out, 16)

            for j in range(4):
                copy_j(j)
            store_m(1)
            copy_j(4)
            store_m(3)
            copy_j(5)
            store_m(5)
            copy_j(6)
            copy_j(7)
            # m=7 h=1 (group 15, odd) — the kernel's final store
            scalar.wait_ge(v_sem, 8)
            scalar.dma_start(
                out_r[7][:, NF : 2 * NF], ot_sb[:, 15 * NF : 16 * NF]
            ).then_inc(dma_out, 16)

        @block.gpsimd
        def _(gpsimd):
            gpsimd.dma_start(ncsq_sb[:], ncsq).then_inc(const_sem, 16)
            gpsimd.dma_start(nxsq_sb[:], nxsq).then_inc(const_sem, 16)
            gpsimd.wait_ge(const_sem, 32)
            # even-group epilogue stage 2: add the -xsq row from staging
            # (the -csq term was already added by Act's bias)
            for j in range(8):
                g = 2 * j
                _, h = _g_mh(g)
                gpsimd.wait_ge(a_sem, j + 1)
                nc.gpsimd.tensor_tensor(
                    ot_sb[:, g * NF : (g + 1) * NF],
                    tmp_sb[:, (j % 2) * NF : (j % 2 + 1) * NF],
                    nxsq_sb[:, h * NF : (h + 1) * NF],
                    op=mybir.AluOpType.add,
                ).then_inc(g_sem, 1)

        @block.tensor
        def _(tensor):
            # warm-up: open the HAM clock gate while the loads stream.
            # wu_sb is deliberately uninitialized — only PE-busy time
            # matters; bank 7 is rewritten with start=True by group 7.
            for _ in range(N_WU):
                nc.tensor.matmul(
                    ps[GP1 - 1][:],
                    wu_sb[:, :, 0:P],
                    wu_sb[:],
                    start=True,
                    stop=True,
                    perf_mode=mybir.MatmulPerfMode.DoubleRow,
                )
            # pass 1: groups 0-7 accumulate in banks 0-7, d outermost so
            # matmuls pace with the streaming loads
            for d in range(D_TILES):
                tensor.wait_ge(d_sems[d], 32)
                for g in range(GP1):
                    m, h = _g_mh(g)
                    mm = nc.tensor.matmul(
                        ps[g][:],
                        ct_sb[d][:, :, m * P : (m + 1) * P],
                        xt_sb[d][:, :, h * NF : (h + 1) * NF],
                        start=(d == 0),
                        stop=(d == D_TILES - 1),
                        perf_mode=mybir.MatmulPerfMode.DoubleRow,
                    )
                    if d == D_TILES - 1:
                        mm.then_inc(mm_sem, 1)
            # pass 2: groups 8-15 reuse banks 0-7 once the epilogue has
            # drained the pass-1 group from that bank
            for g in range(GP1, G):
                m, h = _g_mh(g)
                epi_wait(tensor, g - 8)
                for d in range(D_TILES):
                    mm = nc.tensor.matmul(
                        ps[g % 8][:],
                        ct_sb[d][:, :, m * P : (m + 1) * P],
                        xt_sb[d][:, :, h * NF : (h + 1) * NF],
                        start=(d == 0),
                        stop=(d == D_TILES - 1),
                        perf_mode=mybir.MatmulPerfMode.DoubleRow,
                    )
                mm.then_inc(mm_sem, 1)

        @block.vector
        def _(vector):
            vector.wait_ge(const_sem, 32)  # ncsq + nxsq present
            for g in range(1, G, 2):
                m, h = _g_mh(g)
                vector.wait_ge(mm_sem, g + 1)
                nc.vector.scalar_tensor_tensor(
                    ot_sb[:, g * NF : (g + 1) * NF],
                    ps[g % 8][:],
                    ncsq_sb[:, m : m + 1],
                    nxsq_sb[:, h * NF : (h + 1) * NF],
                    op0=mybir.AluOpType.add,
                    op1=mybir.AluOpType.add,
                ).then_inc(v_sem, 1)

    nc.compile()
    return nc


def _get_nc():
    if not hasattr(_cache, "nc"):
        _cache.nc = _build_nc()
    return _cache.nc


def _to_double_row(a):
    """[D, F] -> [D_TILES, P, DR*F] with d = t*256 + i*128 + p."""
    f = a.shape[1]
    return np.ascontiguousarray(
        a.reshape(D_TILES, DR, P, f).transpose(0, 2, 1, 3).reshape(D_TILES, P, DR * f)
    )


def kernel(inputs, centers, _trace=False):
    inputs = np.asarray(inputs, dtype=np.float32)
    centers = np.asarray(centers, dtype=np.float32)

    csq = np.sum(centers.astype(np.float64) ** 2, axis=1)
    xsq = np.sum(inputs.astype(np.float64) ** 2, axis=1)

    ct = _to_double_row(np.ascontiguousarray(centers.T).astype(_NP_DT))
    xt2 = np.ascontiguousarray((2.0 * inputs).T.astype(_NP_DT))
    ncsq = np.ascontiguousarray((-csq).reshape(M_TILES, P).T.astype(np.float32))

    in_maps = []
    for i in range(N_CORES):
        sl = slice(i * NSH, (i + 1) * NSH)
        in_maps.append(
            {
                "ct": ct,
                "xt": _to_double_row(np.ascontiguousarray(xt2[:, sl])),
                "ncsq": ncsq,
                "nxsq": np.ascontiguousarray(
                    np.broadcast_to(-xsq[sl], (P, NSH))
                ).astype(np.float32),
            }
        )

    nc = _get_nc()
    try:
        res = run_bass_kernel_spmd(
            nc, in_maps, core_ids=list(range(N_CORES)), trace=_trace
        )
    except ModuleNotFoundError:
        # NTFF trace glue is absent in some images; rerun without tracing
        res = run_bass_kernel_spmd(
            nc, in_maps, core_ids=list(range(N_CORES)), trace=False
        )
    if _trace:
        kernel.last_results = res
    return np.concatenate(
        [r["out"].astype(np.float32) for r in res.results], axis=1
    )


# revision 19
# speedup vs baseline: 1.4238x; 1.1495x over previous
"""Trainium2 Bass kernel for nn_CentersDistance (retrieval_knn).

logits[k, n] = -||centers[k] - inputs[n]||^2
             = 2*(centers @ inputs.T)[k, n] - ||centers[k]||^2 - ||inputs[n]||^2

Strategy (8 NeuronCores, data-parallel over the N=8192 inputs):
  * host: transpose both operands so the contraction dim D lands on the SBUF
    partition axis, fold the factor 2 into the inputs, quantize both to
    fp8e4m3, and precompute the norm terms exactly in float64.
  * device (per core): a 1024x1024x1024 matmul in fp8 DoubleRow mode
    (2 contraction rows/cycle on the PE = 157 TF/s, 2x the bf16 rate).
    DoubleRow packs two contraction sub-rows per partition: operands are
    laid out [128, 2, free] per 256-deep d-super-tile (4 tiles cover
    D=1024), so the whole per-core product is 64 matmuls x 512 moving
    rows = 32768 PE cycles = 13.7 us of PE stream.
  * epilogue adds the exact norm terms (-csq per-partition scalar, -xsq
    broadcast row) with scalar_tensor_tensor, split across the DVE
    (even groups) and Pool/GpSimd (odd groups) engines so the ~0.7 us
    per-group PSUM-read cost never becomes the critical path; output is
    written fp16 (the norm terms dominate the logits, measured absmax
    error stays ~5e-3 of scale) and upconverted to fp32 on the host.
    fp16 stores also halve the output DMA traffic: all queues share the
    same 16 DMA engines (~368 GB/s per core total), so with fp8 loads
    (2 MB) + fp16 stores (2 MB) + norm tiles the total DMA time stays
    under the PE stream time.
  * raw Block/semaphore implementation (not Tile), same skeleton as the
    earlier bf16 version: PE warmup matmuls open the HAM clock gate
    while loads stream; pass 1 (m-tiles 0-3) runs d outermost to pace
    with the streaming loads across 8 PSUM banks; pass 2 (m-tiles 4-7)
    runs d innermost so groups retire early and their epilogue + store
    overlap the remaining matmuls.  Bank reuse in pass 2 waits on the
    corresponding epilogue (concurrent PE-write + DVE-read of a PSUM
    bank is fatal on P10).
  * loads stream on two HW-DGE queues (Sync: xt, Scalar: ct) with one
    semaphore per d-tile pair; the norm tiles ride the GpSimd queue.
    Stores go out one m-tile (two groups, [128, 1024] fp16 = 256 KB) at
    a time, even m on Sync, odd m on Scalar, final m-tile split so the
    two halves land on both queues.

Previous bf16 version measured 44.2 us NEFF exec; ~27.6 us of that was
the bf16 PE-stream floor, plus ~8.5 us fixed NRT pre/postamble
(51-semaphore reset chains per engine) that this version keeps paying.
"""

import threading
from contextlib import ExitStack

import numpy as np
import ml_dtypes

import concourse.mybir as mybir
from concourse import bacc
from concourse.bass_utils import run_bass_kernel_spmd

N_CORES = 8
N, K, D = 8192, 1024, 1024
NSH = N // N_CORES  # per-core slab of inputs
P = 128             # SBUF partitions
NF = 512            # matmul moving free dim (one fp32 PSUM bank)

DR = 2              # DoubleRow: contraction sub-rows per partition
DT_SUPER = P * DR   # 256 contraction rows per d-super-tile
D_TILES = D // DT_SUPER  # 4 contraction super-tiles
M_TILES = K // P    # 8 center tiles
H_TILES = NSH // NF # 2 moving-dim tiles

G = M_TILES * H_TILES  # 16 output groups of [128, 512]
GP1 = 8                # groups 0-7 -> pass 1 (m-tiles 0-3), banks 0-7
N_WU = 10              # PE warm-up matmuls

_DT = mybir.dt.float8e4
_NP_DT = ml_dtypes.float8_e4m3
_OUT_DT = mybir.dt.float16

_cache = threading.local()


def _g_mh(g):
    return g // H_TILES, g % H_TILES


def _build_nc():
    nc = bacc.Bacc(
        "TRN2", target_bir_lowering=False, debug=False, num_devices=N_CORES
    )
    # host pre-interleaved DoubleRow layouts: [t, p, (m|h, i, free)] with
    # logical contraction index d = t*256 + i*128 + p.  The per-matmul
    # operand block [2, 128|512] is CONTIGUOUS within each partition so
    # the LDWEIGHTS/moving APs are simple 2-level patterns.
    ct = nc.dram_tensor("ct", [D_TILES, P, DR * K], _DT, kind="ExternalInput").ap()
    xt = nc.dram_tensor("xt", [D_TILES, P, DR * NSH], _DT, kind="ExternalInput").ap()
    ncsq = nc.dram_tensor(
        "ncsq", [P, M_TILES], mybir.dt.float32, kind="ExternalInput"
    ).ap()
    nxsq = nc.dram_tensor(
        "nxsq", [P, NSH], mybir.dt.float32, kind="ExternalInput"
    ).ap()
    out = nc.dram_tensor("out", [K, NSH], _OUT_DT, kind="ExternalOutput").ap()

    out_r = out.rearrange("(m p) n -> m p n", p=P)

    def _ct_op(ct_sb_d, m):
        """[128, 2, 128] contiguous stationary block for m-tile m."""
        return ct_sb_d[:, m * DR * P : (m + 1) * DR * P].rearrange(
            "p (i k) -> p i k", i=DR
        )

    def _xt_op(xt_sb_d, h):
        """[128, 2, 512] contiguous moving block for h-tile h."""
        return xt_sb_d[:, h * DR * NF : (h + 1) * DR * NF].rearrange(
            "p (i n) -> p i n", i=DR
        )

    with (
        nc.sbuf_tensor("wu_sb", [P, DR * NF], _DT) as wu_sb,
        nc.sbuf_tensor("ncsq_sb", [P, M_TILES], mybir.dt.float32) as ncsq_sb,
        nc.sbuf_tensor("nxsq_sb", [P, NSH], mybir.dt.float32) as nxsq_sb,
        nc.sbuf_tensor("ot_sb", [P, G * NF], _OUT_DT) as ot_sb,
        # double-buffered staging for the Act->GpSimd even-group pipeline
        nc.sbuf_tensor("tmp_sb", [P, 2 * NF], _OUT_DT) as tmp_sb,
        ExitStack() as stack,
        nc.semaphore("const_sem") as const_sem,
        nc.semaphore("mm_sem") as mm_sem,
        nc.semaphore("v_sem") as v_sem,   # odd-group epilogues (DVE, from PSUM)
        nc.semaphore("a_sem") as a_sem,   # even-group PSUM->SBUF copies (Act)
        nc.semaphore("g_sem") as g_sem,   # even-group epilogues (GpSimd, from SBUF)
        nc.semaphore("dma_out") as dma_out,
        nc.Block() as block,
    ):
        d_sems = [
            stack.enter_context(nc.semaphore(f"d_sem{i}")) for i in range(D_TILES)
        ]
        ct_sb = [
            stack.enter_context(nc.sbuf_tensor(f"ct_sb{d}", [P, DR * K], _DT))
            for d in range(D_TILES)
        ]
        xt_sb = [
            stack.enter_context(nc.sbuf_tensor(f"xt_sb{d}", [P, DR * NSH], _DT))
            for d in range(D_TILES)
        ]
        ps = [
            stack.enter_context(nc.psum_tensor(f"ps{b}", [P, NF], mybir.dt.float32))
            for b in range(8)
        ]

        def epi_wait(engine, g):
            """Wait until the epilogue for group g has drained its bank."""
            if g % 2 == 0:
                engine.wait_ge(a_sem, g // 2 + 1)
            else:
                engine.wait_ge(v_sem, g // 2 + 1)

        @block.sync
        def _(sync):
            for d in range(D_TILES):
                sync.dma_start(xt_sb[d][:], xt[d]).then_inc(d_sems[d], 16)
            # nxsq rides the same queue AFTER the xt tiles so it does not
            # contend with the PE-pacing loads; first consumer is the
            # first epilogue at ~pass-1 end
            sync.dma_start(nxsq_sb[:], nxsq).then_inc(const_sem, 16)
            # stores: one m-tile = groups 2m, 2m+1 = [128, 1024] fp16
            for m in (0, 2, 4):
                sync.wait_ge(g_sem, m + 1)
                sync.wait_ge(v_sem, m + 1)
                sync.dma_start(
                    out_r[m], ot_sb[:, 2 * m * NF : (2 * m + 2) * NF]
                ).then_inc(dma_out, 16)
            sync.wait_ge(v_sem, 8)  # groups 12, 13 (both on DVE)
            sync.dma_start(
                out_r[6], ot_sb[:, 12 * NF : 14 * NF]
            ).then_inc(dma_out, 16)
            # m=7 h=0 (group 14, on DVE) as soon as its epilogue lands
            sync.wait_ge(v_sem, 9)
            sync.dma_start(
                out_r[7][:, 0:NF], ot_sb[:, 14 * NF : 15 * NF]
            ).then_inc(dma_out, 16)
            sync.wait_ge(dma_out, 9 * 16)

        @block.scalar
        def _(scalar):
            for d in range(D_TILES):
                scalar.dma_start(ct_sb[d][:], ct[d]).then_inc(d_sems[d], 16)
            # even-group epilogue stage 1: Act drains the PSUM bank into
            # fp16 staging (GpSimd cannot read PSUM on TRN2); the odd-m
            # stores are interleaved in expected-readiness order so they
            # are not program-order-blocked behind late copies
            def copy_j(j):
                g = 2 * j
                m, _ = _g_mh(g)
                scalar.wait_ge(mm_sem, g + 1)
                if j >= 2:
                    scalar.wait_ge(g_sem, j - 1)  # staging slot free
                # drain the bank adding the per-partition -csq on the way
                nc.scalar.activation(
                    tmp_sb[:, (j % 2) * NF : (j % 2 + 1) * NF],
                    ps[g % 8][:],
                    mybir.ActivationFunctionType.Identity,
                    bias=ncsq_sb[:, m : m + 1],
                    scale=1.0,
                ).then_inc(a_sem, 1)

            def store_m(m):
                scalar.wait_ge(g_sem, m + 1)
                scalar.wait_ge(v_sem, m + 1)
                scalar.dma_start(
                    out_r[m], ot_sb[:, 2 * m * NF : (2 * m + 2) * NF]
                ).then_inc(dma_out, 16)

            for j in range(4):
                copy_j(j)
            store_m(1)
            copy_j(4)
            store_m(3)
            copy_j(5)
            store_m(5)
            # m=7 h=1 (group 15, on DVE) — the kernel's final store
            scalar.wait_ge(v_sem, 10)
            scalar.dma_start(
                out_r[7][:, NF : 2 * NF], ot_sb[:, 15 * NF : 16 * NF]
            ).then_inc(dma_out, 16)

        @block.gpsimd
        def _(gpsimd):
            gpsimd.dma_start(ncsq_sb[:], ncsq).then_inc(const_sem, 16)
            gpsimd.wait_ge(const_sem, 32)
            # even-group epilogue stage 2: add the -xsq row from staging
            # (the -csq term was already added by Act's bias)
            for j in range(6):
                g = 2 * j
                _, h = _g_mh(g)
                gpsimd.wait_ge(a_sem, j + 1)
                nc.gpsimd.tensor_tensor(
                    ot_sb[:, g * NF : (g + 1) * NF],
                    tmp_sb[:, (j % 2) * NF : (j % 2 + 1) * NF],
                    nxsq_sb[:, h * NF : (h + 1) * NF],
                    op=mybir.AluOpType.add,
                ).then_inc(g_sem, 1)

        @block.tensor
        def _(tensor):
            # warm-up: open the HAM clock gate while the loads stream.
            # wu_sb is deliberately uninitialized — only PE-busy time
            # matters; bank 7 is rewritten with start=True by group 7.
            for _ in range(N_WU):
                nc.tensor.matmul(
                    ps[GP1 - 1][:],
                    _ct_op(wu_sb, 0),
                    _xt_op(wu_sb, 0),
                    start=True,
                    stop=True,
                    perf_mode=mybir.MatmulPerfMode.DoubleRow,
                )
            # pass 1: groups 0-7 accumulate in banks 0-7, d outermost so
            # matmuls pace with the streaming loads
            for d in range(D_TILES):
                tensor.wait_ge(d_sems[d], 32)
                for g in range(GP1):
                    m, h = _g_mh(g)
                    mm = nc.tensor.matmul(
                        ps[g][:],
                        _ct_op(ct_sb[d], m),
                        _xt_op(xt_sb[d], h),
                        start=(d == 0),
                        stop=(d == D_TILES - 1),
                        perf_mode=mybir.MatmulPerfMode.DoubleRow,
                    )
                    if d == D_TILES - 1:
                        mm.then_inc(mm_sem, 1)
            # pass 2: groups 8-15 reuse banks 0-7 once the epilogue has
            # drained the pass-1 group from that bank
            for g in range(GP1, G):
                m, h = _g_mh(g)
                epi_wait(tensor, g - 8)
                for d in range(D_TILES):
                    mm = nc.tensor.matmul(
                        ps[g % 8][:],
                        _ct_op(ct_sb[d], m),
                        _xt_op(xt_sb[d], h),
                        start=(d == 0),
                        stop=(d == D_TILES - 1),
                        perf_mode=mybir.MatmulPerfMode.DoubleRow,
                    )
                mm.then_inc(mm_sem, 1)

        @block.vector
        def _(vector):
            vector.wait_ge(const_sem, 32)  # ncsq + nxsq present
            # odd groups plus the four tail groups 12-15: the DVE reads
            # PSUM directly (~0.75us/group), so the kernel tail is two
            # back-to-back DVE ops instead of the slower Act->Pool chain
            for g in (1, 3, 5, 7, 9, 11, 12, 13, 14, 15):
                m, h = _g_mh(g)
                vector.wait_ge(mm_sem, g + 1)
                nc.vector.scalar_tensor_tensor(
                    ot_sb[:, g * NF : (g + 1) * NF],
                    ps[g % 8][:],
                    ncsq_sb[:, m : m + 1],
                    nxsq_sb[:, h * NF : (h + 1) * NF],
                    op0=mybir.AluOpType.add,
                    op1=mybir.AluOpType.add,
                ).then_inc(v_sem, 1)

    nc.compile()
    return nc


def _get_nc():
    if not hasattr(_cache, "nc"):
        _cache.nc = _build_nc()
    return _cache.nc


def _to_double_row(a, blk):
    """[D, F] -> [D_TILES, P, DR*F] with d = t*256 + i*128 + p and the
    free axis grouped as (block, i, f%blk) so each per-matmul operand
    block [DR, blk] is contiguous within a partition."""
    f = a.shape[1]
    return np.ascontiguousarray(
        a.reshape(D_TILES, DR, P, f // blk, blk)
        .transpose(0, 2, 3, 1, 4)
        .reshape(D_TILES, P, DR * f)
    )


def kernel(inputs, centers, _trace=False):
    inputs = np.asarray(inputs, dtype=np.float32)
    centers = np.asarray(centers, dtype=np.float32)

    csq = np.sum(centers.astype(np.float64) ** 2, axis=1)
    xsq = np.sum(inputs.astype(np.float64) ** 2, axis=1)

    ct = _to_double_row(np.ascontiguousarray(centers.T).astype(_NP_DT), P)
    xt2 = np.ascontiguousarray((2.0 * inputs).T.astype(_NP_DT))
    ncsq = np.ascontiguousarray((-csq).reshape(M_TILES, P).T.astype(np.float32))

    in_maps = []
    for i in range(N_CORES):
        sl = slice(i * NSH, (i + 1) * NSH)
        in_maps.append(
            {
                "ct": ct,
                "xt": _to_double_row(np.ascontiguousarray(xt2[:, sl]), NF),
                "ncsq": ncsq,
                "nxsq": np.ascontiguousarray(
                    np.broadcast_to(-xsq[sl], (P, NSH))
                ).astype(np.float32),
            }
        )

    nc = _get_nc()
    try:
        res = run_bass_kernel_spmd(
            nc, in_maps, core_ids=list(range(N_CORES)), trace=_trace
        )
    except ModuleNotFoundError:
        # NTFF trace glue is absent in some images; rerun without tracing
        res = run_bass_kernel_spmd(
            nc, in_maps, core_ids=list(range(N_CORES)), trace=False
        )
    if _trace:
        kernel.last_results = res
    return np.concatenate(
        [r["out"].astype(np.float32) for r in res.results], axis=1
    )
